# revision 2
# baseline (speedup 1.0000x reference)
"""Trainium2 Bass kernel for 4M per-element tiny MLPs (1->8->1, H=8).

    y[i] = W2[i] @ relu(W1[i] * x[i] + b1[i]) + b2[i]

Memory-bound; data-parallel over 8 NeuronCores (500k nets/core), no
communication.

v2 design (on top of the fp16 j-major slab baseline, which measured
104.8 us DMA-period-bound: DMA active 81.7 us > DVE 74.9 us > ACT 39.6):
  * W1 is stored in DRAM as fp8 e3m4 (TRN float8e3; 4 mantissa bits,
    values in (-1,1) so no overflow and e3m4 beats e4m3 2x on precision;
    measured end-to-end rel_l2 7.5e-3 vs the 2e-2 gate). Slab shrinks
    52 -> 44 B/net (27 -> 23 MB/core), pushing DMA below DVE.
  * The fp8->fp16 upconvert runs on the ACT (scalar) engine, which has
    ~35 us of slack: per tile ACT does upconvert(t) [~2 us] + relu(t)
    [~2 us] inside the ~5.2 us DVE period. DVE op mix is unchanged
    (all operands it touches stay fp16 at 2x_1p).
  * Slab is a single uint8 DMA per tile; sections are bitcast views:
    [W1 f8e3 j-major 8fi B | b1 f16 j-major 16fi B | W2 f16 j-major
     16fi B | x f16 2fi B | b2 f16 2fi B] = 44fi B/partition.
"""

import numpy as np
import ml_dtypes
from contextlib import ExitStack

import concourse.bacc as bacc
import concourse.mybir as mybir
import concourse.tile as tile
from concourse.bass_utils import run_bass_kernel_spmd

F16 = mybir.dt.float16
F8 = mybir.dt.float8e3
U8 = mybir.dt.uint8
AF = mybir.ActivationFunctionType
OP = mybir.AluOpType
E3M4 = ml_dtypes.float8_e3m4

N = 4_000_000
H = 8
N_CORES = 8
R = N // N_CORES            # 500,000 nets per core
FP = 3907                   # nets per partition (padded): 128*3907 = 500,096
R_PAD = 128 * FP
FIS = [32, 256] + [288] * 12 + [163]   # small first tile primes the pipeline
SLAB_B = 44                 # bytes per net in the slab: 8(W1 f8) + 16 + 16 + 2 + 2


def build_nc(fis):
    fp = sum(fis)
    rp = 128 * fp

    nc = bacc.Bacc("TRN2", target_bir_lowering=False, debug=False)

    slab = nc.dram_tensor("slab", [rp * SLAB_B], U8, kind="ExternalInput")
    ys = nc.dram_tensor("ys", [rp], F16, kind="ExternalOutput")

    with tile.TileContext(nc) as tc, ExitStack() as ctx, \
            nc.allow_low_precision(reason="fp16/fp8 kernel, tol 2e-2"):
        spool = ctx.enter_context(tc.tile_pool(name="s", bufs=5))
        wpool = ctx.enter_context(tc.tile_pool(name="w", bufs=2))
        zpool = ctx.enter_context(tc.tile_pool(name="z", bufs=2))
        vpool = ctx.enter_context(tc.tile_pool(name="v", bufs=2))

        def emit_tail(fi, rb, zc, w2v, b2v):
            zd = zpool.tile([128, H * fi], F16, tag="zd")
            nc.vector.tensor_tensor(zd[:], zc[:], w2v, op=OP.mult)
            u1 = vpool.tile([128, 4 * fi], F16, tag="u1")
            nc.vector.tensor_tensor(
                u1[:], zd[:, 0:4 * fi], zd[:, 4 * fi:8 * fi], op=OP.add
            )
            u2 = vpool.tile([128, 2 * fi], F16, tag="u2")
            nc.vector.tensor_tensor(
                u2[:], u1[:, 0:2 * fi], u1[:, 2 * fi:4 * fi], op=OP.add
            )
            yt = vpool.tile([128, fi], F16, tag="yt")
            nc.vector.tensor_tensor(yt[:], u2[:, 0:fi], u2[:, fi:2 * fi], op=OP.add)
            yo = vpool.tile([128, fi], F16, tag="yo")
            nc.vector.tensor_tensor(yo[:], yt[:], b2v, op=OP.add)
            nc.scalar.dma_start(
                ys.ap()[rb:rb + 128 * fi].rearrange("(p f) -> p f", p=128), yo[:]
            )

        prev = None
        rb = 0
        for ti, fi in enumerate(fis):
            nrows = 128 * fi
            S = spool.tile([128, SLAB_B * fi], U8, tag="slab")
            src = slab.ap()[rb * SLAB_B:(rb + nrows) * SLAB_B].rearrange(
                "(p k) -> p k", p=128
            )
            (nc.sync if ti % 2 == 0 else nc.scalar).dma_start(S[:], src)

            w1f8 = S[:, 0:8 * fi].bitcast(F8)
            b1v = S[:, 8 * fi:24 * fi].bitcast(F16)
            w2v = S[:, 24 * fi:40 * fi].bitcast(F16)
            xv = S[:, 40 * fi:42 * fi].bitcast(F16)
            b2v = S[:, 42 * fi:44 * fi].bitcast(F16)
            xb = xv.rearrange("p (o f) -> p o f", o=1).broadcast_to([128, H, fi])

            w1 = wpool.tile([128, H * fi], F16, tag="w1")
            nc.scalar.copy(w1[:], w1f8)
            w1v = w1[:].rearrange("p (j f) -> p j f", j=H)

            za = zpool.tile([128, H * fi], F16, tag="za")
            zb = zpool.tile([128, H * fi], F16, tag="zb")
            zc = zpool.tile([128, H * fi], F16, tag="zc")

            nc.vector.tensor_tensor(
                za[:].rearrange("p (j f) -> p j f", j=H), xb, w1v, op=OP.mult
            )
            nc.vector.tensor_tensor(zb[:], za[:], b1v, op=OP.add)
            nc.scalar.activation(zc[:], zb[:], AF.Relu)

            if prev is not None:
                emit_tail(*prev)
            prev = (fi, rb, zc, w2v, b2v)
            rb += nrows
        emit_tail(*prev)

    nc.compile()
    return nc


# ---------------- entry point ----------------

_CACHE = {}


def _get_nc():
    if "nc" not in _CACHE:
        _CACHE["nc"] = build_nc(FIS)
    return _CACHE["nc"]


def _pack_core(w1u8, b1, w2, xs, b2):
    """Build the interleaved j-major mixed-dtype slab for one core.

    Inputs are padded per-core arrays: w1u8 [R_PAD, 8] uint8 (e3m4
    bytes), b1/w2 [R_PAD, 8] fp16, xs/b2 [R_PAD] fp16. Tile t (fi
    nets/partition): net = rb + p*fi + f. Slab tile = [128, 44*fi] B:
    [W1 f8 j-major | b1 j-major | W2 j-major | x | b2].
    """
    parts = []
    rb = 0
    for fi in FIS:
        nrows = 128 * fi
        jmaj = lambda a: np.ascontiguousarray(
            a[rb:rb + nrows].reshape(128, fi, H).transpose(0, 2, 1)
        ).reshape(128, H * fi)
        t = np.concatenate(
            [
                jmaj(w1u8),
                jmaj(b1).view(np.uint8),
                jmaj(w2).view(np.uint8),
                xs[rb:rb + nrows].reshape(128, fi).view(np.uint8),
                b2[rb:rb + nrows].reshape(128, fi).view(np.uint8),
            ],
            axis=1,
        )
        parts.append(t.reshape(-1))
        rb += nrows
    return np.concatenate(parts)


def _pad2(a, dt):
    out = np.zeros((R_PAD, H), dt)
    out[:R] = a
    return out


def _pad1(a):
    out = np.zeros(R_PAD, np.float16)
    out[:R] = a
    return out


def _make_in_maps(x, W1, b1, W2, b2):
    x = np.asarray(x, np.float16)
    W1u8 = np.asarray(W1, np.float32).astype(E3M4).view(np.uint8)
    b1 = np.asarray(b1, np.float16)
    W2 = np.asarray(W2, np.float16)
    b2 = np.asarray(b2, np.float16)
    in_maps = []
    for c in range(N_CORES):
        sl = slice(c * R, (c + 1) * R)
        in_maps.append({
            "slab": _pack_core(
                _pad2(W1u8[sl], np.uint8), _pad2(b1[sl], np.float16),
                _pad2(W2[sl], np.float16),
                _pad1(x[sl, 0]), _pad1(b2[sl, 0]),
            ),
        })
    return in_maps


def _run(x, W1, b1, W2, b2, **kw):
    nc = _get_nc()
    res = run_bass_kernel_spmd(nc, _make_in_maps(x, W1, b1, W2, b2),
                               core_ids=list(range(N_CORES)), **kw)
    y = np.empty((N, 1), np.float32)
    for c in range(N_CORES):
        y[c * R:(c + 1) * R, 0] = res.results[c]["ys"].reshape(-1)[:R].astype(
            np.float32
        )
    return y, res


def kernel(x, W1, b1, W2, b2):
    y, _ = _run(x, W1, b1, W2, b2)
    return y
